# revision 26
# baseline (speedup 1.0000x reference)
"""LogisticMapDenseLayer Trainium2 kernel.

Reference computation (see problem):
    r_mapped = 3.57 + (4.0 - 3.57) * r
    w[i+1] = r_mapped * w[i] * (1 - w[i])   (NW = 512*512 sequential steps)
    out = x @ w.reshape(512, 512) + bias    (x: [32768, 512])

The chaotic scalar recurrence is inherently serial, so it is computed once on
the host with bit-exact fp32 arithmetic, and the memory-bound batch matmul is
data-parallel sharded across the 8 NeuronCores.

Device-side design (per core, B_SH = 4096 rows):
  - fp16 end-to-end I/O: x is pre-transposed/packed and cast to fp16 on the
    host, w cast to fp16, y written back fp16 and up-converted (+bias) on
    the host. fp16 matmul runs at the same 1 cycle/row as f32r on the PE,
    so this halves HBM traffic at no PE cost; accumulation stays fp32 in
    PSUM (rel-err ~3e-4, gate is 2e-2).
  - All DRAM layouts are packed so every DMA is 128 descriptors (one
    contiguous run per SBUF partition). A DMA's completion semaphore is
    bumped per DMA-engine and the last increment lags ~2us for 512-desc
    transfers vs ~0.9us for 128-desc ones; packing cuts both the pipeline
    head (first chunk ready sooner) and the tail (last y store completes
    sooner).
  - Fine-grained chunking (1..4 m-tiles per DMA): small first chunks sized
    so the PE never stalls once real matmuls begin, single-tile last
    chunks for a short writeback tail.
  - x/w load on the SP (sync) HWDGE ring; y stores on the Activation ring;
    the last two PSUM evictions run on ACT too (same engine as the store)
    while the rest run on the DVE.
  - PE power warmup: the HAM duty-cycle governor holds the PE at 50% duty
    and boosts to 100% only after ~3.5us of SUSTAINED high power draw
    (idle resets it; low-entropy data draws too little). A burst of
    matmuls on raw random bits bridges directly into the real stream so
    the boost lands just as real data arrives.
"""
import os
import sys
import types
from contextlib import ExitStack

import numpy as np

# ---------------------------------------------------------------- constants
B, D, U = 32768, 512, 512
NW = D * U
R_LO, R_HI = 3.57, 4.0
N_CORES = 8
B_SH = B // N_CORES          # 4096 rows per core
P = 128
KO = D // P                  # 4 contraction chunks
N_TILES = B_SH // P          # 32 m-tiles per core
CHUNKS = (2, 2, 4, 4, 4, 4, 4, 4, 2, 1, 1)
assert sum(CHUNKS) == N_TILES
N_WARMUP = int(os.environ.get("LMAP_WARMUP", "12"))
RAW_RNG = os.environ.get("LMAP_RAWRNG", "1") == "1"
VARIANT = "f16-packed"   # informational (test.py prints it)


def _install_ntff_shim():
    """antenv.axon_hooks is absent in this image; bass_utils imports it when
    tracing is requested (e.g. BASS_TRACE=1). Provide a working shim backed by
    trn_agent_boot's ctypes NTFF driver; degrade to hook=None on any failure."""
    try:
        import antenv.axon_hooks  # noqa: F401
        return
    except Exception:
        pass
    try:
        import antenv

        mod = types.ModuleType("antenv.axon_hooks")
        mod._hook = None
        try:
            from trn_agent_boot.trn_boot import _ntff_profile_via_ctypes

            mod._hook = _ntff_profile_via_ctypes("/opt/axon/libaxon_pjrt.so")
        except Exception:
            pass
        mod.get_axon_ntff_profile_hook = lambda: mod._hook
        mod.set_axon_ntff_profile_hook = lambda h: setattr(mod, "_hook", h)
        sys.modules["antenv.axon_hooks"] = mod
        antenv.axon_hooks = mod
    except Exception:
        pass


_install_ntff_shim()

import concourse.bass as bass  # noqa: E402
import concourse.mybir as mybir  # noqa: E402
import concourse.tile as tile  # noqa: E402
from concourse import bacc  # noqa: E402
from concourse.bass_utils import run_bass_kernel_spmd  # noqa: E402

F32 = mybir.dt.float32
F16 = mybir.dt.float16


# ---------------------------------------------------------------- host side
def _gen_weights(r: np.float32, x0: np.float32) -> np.ndarray:
    """Bit-exact fp32 logistic-map weight generation (matches the jax scan:
    each step is round32(round32(r_mapped*c) * round32(1-c)))."""
    rm = np.float32(np.float32(R_LO) + np.float32(np.float32(R_HI - R_LO) * r))
    one = np.float32(1.0)
    w = np.empty(NW, dtype=np.float32)
    c = np.float32(x0)
    for i in range(NW):
        c = np.float32(np.float32(rm * c) * np.float32(one - c))
        w[i] = c
    return w.reshape(D, U)


def _pack_x(x_shard: np.ndarray) -> np.ndarray:
    """[B_SH, D] f32 -> packed [128, KO*B_SH] fp16: per chunk, each SBUF
    partition's bytes are one contiguous run (k-major within the chunk)."""
    xt = x_shard.T.astype(np.float16).reshape(KO, P, B_SH)   # [ko, p, b]
    blocks = []
    lo = 0
    for cw in CHUNKS:
        blk = xt[:, :, lo:lo + cw * P]                       # [ko, p, cwP]
        blocks.append(blk.transpose(1, 0, 2).reshape(P, KO * cw * P))
        lo += cw * P
    return np.ascontiguousarray(np.concatenate(blocks, axis=1))


def _pack_w(w16: np.ndarray) -> np.ndarray:
    """[D, U] fp16 -> [128, KO*U] (k-major blocks)."""
    return np.ascontiguousarray(
        w16.reshape(KO, P, U).transpose(1, 0, 2).reshape(P, KO * U))


def _unpack_y(y_dev: np.ndarray) -> np.ndarray:
    """[128, N_TILES*U] fp16 -> [B_SH, U] fp16."""
    return y_dev.reshape(P, N_TILES, U).transpose(1, 0, 2).reshape(B_SH, U)


# ---------------------------------------------------------------- bass side
def _build():
    nc = bacc.Bacc("TRN2", target_bir_lowering=False, debug=False,
                   num_devices=N_CORES)
    xt = nc.dram_tensor("xt", [P, KO * B_SH], F16, kind="ExternalInput").ap()
    w = nc.dram_tensor("w", [P, KO * U], F16, kind="ExternalInput").ap()
    y = nc.dram_tensor("y", [P, N_TILES * U], F16, kind="ExternalOutput").ap()

    with ExitStack() as ctx:
        tc = ctx.enter_context(tile.TileContext(nc))
        wpool = ctx.enter_context(tc.tile_pool(name="wp", bufs=1))
        warm = ctx.enter_context(tc.tile_pool(name="warm", bufs=1))
        xpool = ctx.enter_context(tc.tile_pool(name="xp", bufs=6))
        opool = ctx.enter_context(tc.tile_pool(name="op", bufs=3))
        ps_w = ctx.enter_context(tc.tile_pool(name="ps_w", bufs=1, space="PSUM"))
        ps_o = ctx.enter_context(tc.tile_pool(name="ps_o", bufs=6, space="PSUM"))

        # Loads on the (pre-warmed) SP ring: w first (it gates every k-step
        # of the very first tile), then the x chunks. Chunk sizes are
        # matched to the queue bandwidth so the PE never stalls once the
        # first tile starts: a stall drops the HAM duty boost and costs
        # ~3.5us to win back.
        w_sb = wpool.tile([P, KO * U], F16, tag="w_sb")
        nc.sync.dma_start(w_sb[:], w[:])

        xgs = []
        for ci, cw in enumerate(CHUNKS):
            off = KO * P * sum(CHUNKS[:ci])
            xg = xpool.tile([P, KO * cw * P], F16, tag="xg")
            nc.sync.dma_start(xg[:], xt[:, off:off + KO * cw * P])
            xgs.append(xg)

        # PE power warmup on random bits (see module docstring). Values may
        # be NaN; they only ever reach the dedicated wu_ps PSUM bank, which
        # is never read. LMAP_RAWRNG=0 masks fp16 exponent bit 10 instead
        # (no NaN/Inf bit patterns, but lower power -> later boost).
        if N_WARMUP:
            wu = warm.tile([P, U], mybir.dt.int16, tag="wu")
            nc.vector.random(wu[:])
            if not RAW_RNG:
                nc.vector.tensor_scalar(wu[:], wu[:], 0xFBFF, None,
                                        mybir.AluOpType.bitwise_and)
            wu16 = wu[:].bitcast(F16)
            wu_ps = ps_w.tile([P, U], F32, tag="wu_ps")
            for i in range(N_WARMUP):
                nc.tensor.matmul(wu_ps[:], wu16[:, 0:P], wu16, start=True,
                                 stop=True)

        for ci, cw in enumerate(CHUNKS):
            xg = xgs[ci]
            og = opool.tile([P, cw * U], F16, tag="og")
            for t in range(cw):
                ps_out = ps_o.tile([P, U], F32, tag="ps_out")
                for k in range(KO):
                    nc.tensor.matmul(ps_out[:],
                                     xg[:, (k * cw + t) * P:(k * cw + t + 1) * P],
                                     w_sb[:, k * U:(k + 1) * U],
                                     start=(k == 0), stop=(k == KO - 1))
                # PSUM -> SBUF eviction with fp32 -> fp16 cast on the DVE;
                # the trailing single-tile chunks evict on ACT (same engine
                # as the y-store DMA) for the shortest possible tail.
                # (Splitting each eviction in half across ACT+DVE was tried
                # and is SLOWER: the tile dep-tracker serializes the two
                # half-writes to one og tile as a WAW hazard.)
                if ci >= len(CHUNKS) - 2:
                    nc.scalar.copy(og[:, t * U:(t + 1) * U], ps_out[:])
                else:
                    nc.vector.tensor_copy(og[:, t * U:(t + 1) * U], ps_out[:])
            yoff = U * sum(CHUNKS[:ci])
            # y stores on the ACT ring so reads and writes never share a
            # queue
            nc.scalar.dma_start(y[:, yoff:yoff + cw * U], og[:])
    nc.compile()
    return nc


_NC_CACHE: dict = {}


def _get_nc():
    if "nc" not in _NC_CACHE:
        _NC_CACHE["nc"] = _build()
    return _NC_CACHE["nc"]


# ---------------------------------------------------------------- entry
def kernel(x, r, x0, bias, _trace=False, _trace_cores=None):
    x = np.asarray(x, dtype=np.float32)
    r = np.float32(np.asarray(r))
    x0 = np.float32(np.asarray(x0))
    bias = np.asarray(bias, dtype=np.float32).reshape(U)
    assert x.shape == (B, D)

    w_h = _pack_w(_gen_weights(r, x0).astype(np.float16))

    nc = _get_nc()
    in_maps = [
        {"xt": _pack_x(x[i * B_SH:(i + 1) * B_SH]), "w": w_h}
        for i in range(N_CORES)
    ]
    res = run_bass_kernel_spmd(nc, in_maps, core_ids=list(range(N_CORES)),
                               trace=_trace, trace_cores=_trace_cores)
    out = np.concatenate([_unpack_y(res.results[i]["y"])
                          for i in range(N_CORES)], axis=0).astype(np.float32)
    out += bias[None, :]
    if _trace:
        kernel._last_result = res
    return out


# revision 29
# speedup vs baseline: 1.0274x; 1.0274x over previous
"""LogisticMapDenseLayer Trainium2 kernel.

Reference computation (see problem):
    r_mapped = 3.57 + (4.0 - 3.57) * r
    w[i+1] = r_mapped * w[i] * (1 - w[i])   (NW = 512*512 sequential steps)
    out = x @ w.reshape(512, 512) + bias    (x: [32768, 512])

The chaotic scalar recurrence is inherently serial, so it is computed once on
the host with bit-exact fp32 arithmetic, and the memory-bound batch matmul is
data-parallel sharded across the 8 NeuronCores.

Device-side design (per core, B_SH = 4096 rows):
  - fp16 end-to-end I/O: x is pre-transposed/packed and cast to fp16 on the
    host, w cast to fp16, y written back fp16 and up-converted (+bias) on
    the host. fp16 matmul runs at the same 1 cycle/row as f32r on the PE,
    so this halves HBM traffic at no PE cost; accumulation stays fp32 in
    PSUM (rel-err ~3e-4, gate is 2e-2).
  - All DRAM layouts are packed so every DMA is 128 descriptors (one
    contiguous run per SBUF partition). A DMA's completion semaphore is
    bumped per DMA-engine and the last increment lags ~2us for 512-desc
    transfers vs ~0.9us for 128-desc ones; packing cuts both the pipeline
    head (first chunk ready sooner) and the tail (last y store completes
    sooner).
  - Fine-grained chunking (1..4 m-tiles per DMA): small first chunks sized
    so the PE never stalls once real matmuls begin, single-tile last
    chunks for a short writeback tail.
  - x/w load on the SP (sync) HWDGE ring; y stores on the Activation ring;
    the last two PSUM evictions run on ACT too (same engine as the store)
    while the rest run on the DVE.
  - PE power warmup: the HAM duty-cycle governor holds the PE at 50% duty
    and boosts to 100% only after ~3.5us of SUSTAINED high power draw
    (idle resets it; low-entropy data draws too little). A burst of
    matmuls on raw random bits bridges directly into the real stream so
    the boost lands just as real data arrives.
"""
import os
import sys
import types
from contextlib import ExitStack

import numpy as np

# ---------------------------------------------------------------- constants
B, D, U = 32768, 512, 512
NW = D * U
R_LO, R_HI = 3.57, 4.0
N_CORES = 8
B_SH = B // N_CORES          # 4096 rows per core
P = 128
KO = D // P                  # 4 contraction chunks
N_TILES = B_SH // P          # 32 m-tiles per core
CHUNKS = (2, 2, 4, 4, 4, 4, 4, 4, 2, 1, 1)
assert sum(CHUNKS) == N_TILES
N_WARMUP = int(os.environ.get("LMAP_WARMUP", "9"))
RAW_RNG = os.environ.get("LMAP_RAWRNG", "1") == "1"
VARIANT = "f16-packed"   # informational (test.py prints it)


def _install_ntff_shim():
    """antenv.axon_hooks is absent in this image; bass_utils imports it when
    tracing is requested (e.g. BASS_TRACE=1). Provide a working shim backed by
    trn_agent_boot's ctypes NTFF driver; degrade to hook=None on any failure."""
    try:
        import antenv.axon_hooks  # noqa: F401
        return
    except Exception:
        pass
    try:
        import antenv

        mod = types.ModuleType("antenv.axon_hooks")
        mod._hook = None
        try:
            from trn_agent_boot.trn_boot import _ntff_profile_via_ctypes

            mod._hook = _ntff_profile_via_ctypes("/opt/axon/libaxon_pjrt.so")
        except Exception:
            pass
        mod.get_axon_ntff_profile_hook = lambda: mod._hook
        mod.set_axon_ntff_profile_hook = lambda h: setattr(mod, "_hook", h)
        sys.modules["antenv.axon_hooks"] = mod
        antenv.axon_hooks = mod
    except Exception:
        pass


_install_ntff_shim()

import concourse.bass as bass  # noqa: E402
import concourse.mybir as mybir  # noqa: E402
import concourse.tile as tile  # noqa: E402
from concourse import bacc  # noqa: E402
from concourse.bass_utils import run_bass_kernel_spmd  # noqa: E402

F32 = mybir.dt.float32
F16 = mybir.dt.float16


# ---------------------------------------------------------------- host side
def _gen_weights(r: np.float32, x0: np.float32) -> np.ndarray:
    """Bit-exact fp32 logistic-map weight generation (matches the jax scan:
    each step is round32(round32(r_mapped*c) * round32(1-c)))."""
    rm = np.float32(np.float32(R_LO) + np.float32(np.float32(R_HI - R_LO) * r))
    one = np.float32(1.0)
    w = np.empty(NW, dtype=np.float32)
    c = np.float32(x0)
    for i in range(NW):
        c = np.float32(np.float32(rm * c) * np.float32(one - c))
        w[i] = c
    return w.reshape(D, U)


def _pack_x(x_shard: np.ndarray) -> np.ndarray:
    """[B_SH, D] f32 -> packed [128, KO*B_SH] fp16: per chunk, each SBUF
    partition's bytes are one contiguous run (k-major within the chunk)."""
    xt = x_shard.T.astype(np.float16).reshape(KO, P, B_SH)   # [ko, p, b]
    blocks = []
    lo = 0
    for cw in CHUNKS:
        blk = xt[:, :, lo:lo + cw * P]                       # [ko, p, cwP]
        blocks.append(blk.transpose(1, 0, 2).reshape(P, KO * cw * P))
        lo += cw * P
    return np.ascontiguousarray(np.concatenate(blocks, axis=1))


def _pack_w(w16: np.ndarray) -> np.ndarray:
    """[D, U] fp16 -> [128, KO*U] (k-major blocks)."""
    return np.ascontiguousarray(
        w16.reshape(KO, P, U).transpose(1, 0, 2).reshape(P, KO * U))


def _unpack_y(y_dev: np.ndarray) -> np.ndarray:
    """[128, N_TILES*U] fp16 -> [B_SH, U] fp16."""
    return y_dev.reshape(P, N_TILES, U).transpose(1, 0, 2).reshape(B_SH, U)


# ---------------------------------------------------------------- bass side
def _build():
    nc = bacc.Bacc("TRN2", target_bir_lowering=False, debug=False,
                   num_devices=N_CORES)
    xt = nc.dram_tensor("xt", [P, KO * B_SH], F16, kind="ExternalInput").ap()
    w = nc.dram_tensor("w", [P, KO * U], F16, kind="ExternalInput").ap()
    y = nc.dram_tensor("y", [P, N_TILES * U], F16, kind="ExternalOutput").ap()

    with ExitStack() as ctx:
        tc = ctx.enter_context(tile.TileContext(nc))
        wpool = ctx.enter_context(tc.tile_pool(name="wp", bufs=1))
        warm = ctx.enter_context(tc.tile_pool(name="warm", bufs=1))
        xpool = ctx.enter_context(tc.tile_pool(name="xp", bufs=6))
        opool = ctx.enter_context(tc.tile_pool(name="op", bufs=3))
        ps_w = ctx.enter_context(tc.tile_pool(name="ps_w", bufs=1, space="PSUM"))
        ps_o = ctx.enter_context(tc.tile_pool(name="ps_o", bufs=6, space="PSUM"))

        # Loads on the (pre-warmed) SP ring: w first (it gates every k-step
        # of the very first tile), then the x chunks. Chunk sizes are
        # matched to the queue bandwidth so the PE never stalls once the
        # first tile starts: a stall drops the HAM duty boost and costs
        # ~3.5us to win back.
        w_sb = wpool.tile([P, KO * U], F16, tag="w_sb")
        nc.sync.dma_start(w_sb[:], w[:])

        xgs = []
        for ci, cw in enumerate(CHUNKS):
            off = KO * P * sum(CHUNKS[:ci])
            xg = xpool.tile([P, KO * cw * P], F16, tag="xg")
            nc.sync.dma_start(xg[:], xt[:, off:off + KO * cw * P])
            xgs.append(xg)

        # PE power warmup on random bits (see module docstring). Values may
        # be NaN; they only ever reach the dedicated wu_ps PSUM bank, which
        # is never read. LMAP_RAWRNG=0 masks fp16 exponent bit 10 instead
        # (no NaN/Inf bit patterns, but lower power -> later boost).
        if N_WARMUP:
            if RAW_RNG:
                # f32r operands double the SBUF read width per PE cycle vs
                # fp16 -> more power -> the boost trigger fires sooner
                wu = warm.tile([P, U], F32, tag="wu")
                nc.vector.random(wu[:])
                wu_mm = wu[:].bitcast(mybir.dt.float32r)
            else:
                wu = warm.tile([P, U], mybir.dt.int16, tag="wu")
                nc.vector.random(wu[:])
                nc.vector.tensor_scalar(wu[:], wu[:], 0xFBFF, None,
                                        mybir.AluOpType.bitwise_and)
                wu_mm = wu[:].bitcast(F16)
            wu_ps = ps_w.tile([P, U], F32, tag="wu_ps")
            for i in range(N_WARMUP):
                nc.tensor.matmul(wu_ps[:], wu_mm[:, 0:P], wu_mm, start=True,
                                 stop=True)
            # extra chip power draw on the otherwise-idle Pool engine while
            # the governor's power integral accumulates
            pw = warm.tile([P, U], F32, tag="pw")
            pw2 = warm.tile([P, U], F32, tag="pw2")
            nc.gpsimd.random(pw[:])
            for i in range(6):
                s, dst = (pw, pw2) if i % 2 == 0 else (pw2, pw)
                nc.gpsimd.tensor_copy(dst[:], s[:])

        for ci, cw in enumerate(CHUNKS):
            xg = xgs[ci]
            og = opool.tile([P, cw * U], F16, tag="og")
            for t in range(cw):
                ps_out = ps_o.tile([P, U], F32, tag="ps_out")
                for k in range(KO):
                    nc.tensor.matmul(ps_out[:],
                                     xg[:, (k * cw + t) * P:(k * cw + t + 1) * P],
                                     w_sb[:, k * U:(k + 1) * U],
                                     start=(k == 0), stop=(k == KO - 1))
                # PSUM -> SBUF eviction with fp32 -> fp16 cast on the DVE;
                # the trailing single-tile chunks evict on ACT (same engine
                # as the y-store DMA) for the shortest possible tail.
                # (Splitting each eviction in half across ACT+DVE was tried
                # and is SLOWER: the tile dep-tracker serializes the two
                # half-writes to one og tile as a WAW hazard.)
                if ci == len(CHUNKS) - 1:
                    nc.scalar.copy(og[:, t * U:(t + 1) * U], ps_out[:])
                else:
                    nc.vector.tensor_copy(og[:, t * U:(t + 1) * U], ps_out[:])
            yoff = U * sum(CHUNKS[:ci])
            # y stores on the ACT ring so reads and writes never share a
            # queue -- except the second-to-last chunk, which evicts on DVE
            # and stores via the (long idle) SP ring so the ACT queue at
            # the tail serves only the very last tile
            store_eng = nc.sync if ci == len(CHUNKS) - 2 else nc.scalar
            store_eng.dma_start(y[:, yoff:yoff + cw * U], og[:])
    nc.compile()
    return nc


_NC_CACHE: dict = {}


def _get_nc():
    if "nc" not in _NC_CACHE:
        _NC_CACHE["nc"] = _build()
    return _NC_CACHE["nc"]


# ---------------------------------------------------------------- entry
def kernel(x, r, x0, bias, _trace=False, _trace_cores=None):
    x = np.asarray(x, dtype=np.float32)
    r = np.float32(np.asarray(r))
    x0 = np.float32(np.asarray(x0))
    bias = np.asarray(bias, dtype=np.float32).reshape(U)
    assert x.shape == (B, D)

    w_h = _pack_w(_gen_weights(r, x0).astype(np.float16))

    nc = _get_nc()
    in_maps = [
        {"xt": _pack_x(x[i * B_SH:(i + 1) * B_SH]), "w": w_h}
        for i in range(N_CORES)
    ]
    res = run_bass_kernel_spmd(nc, in_maps, core_ids=list(range(N_CORES)),
                               trace=_trace, trace_cores=_trace_cores)
    out = np.concatenate([_unpack_y(res.results[i]["y"])
                          for i in range(N_CORES)], axis=0).astype(np.float32)
    out += bias[None, :]
    if _trace:
        kernel._last_result = res
    return out
